# revision 15
# baseline (speedup 1.0000x reference)
"""Trainium2 Bass kernel for nn_LlamaMLP (BitLinear-style ternary-quantized MLP).

Reference computation (all f32):
    s_m   = mean(|w_m|)                            (global scalar per weight)
    q_m   = round(clip(w_m / (s_m + eps), -1, 1))  (ternary)
    gate  = x @ (q_g * s_g).T ; up = x @ (q_u * s_u).T
    out   = (gate * up) @ (q_d * s_d).T
        == (s_g*s_u*s_d) * ((x @ q_g.T) * (x @ q_u.T)) @ q_d.T

Strategy: tensor-parallel over the intermediate dim I. Each core gets f16
weight shards and quantizes them on device to ternary fp8e4 (exact). The gate
weights live SBUF-resident; up/down weights go to DRAM fp8 and stream per
block (~22us/block, hidden). Matmuls run bf16 activations x fp8 ternary
weights (mixed dtype, bf16 rate) with f32 PSUM accumulation.

Critical-path engineering:
  - per-matrix pipeline A(w) -> AllReduce(scale) -> B(quantize) ordered gate,
    up, down. A(wg) splits its |w| reduce across DVE and ACT; A(wu)/A(wd)
    run entirely on ACT (activation Abs + accum_out) so they never queue
    behind B quantize work on DVE. Block 0's gate matmuls start ~140us in.
  - the block loop is software-pipelined as [gate(b+1), up(b), down(b)] so
    the PE always has gate work in hand even if up/down weight streams lag;
  - a dummy AllReduce at t=0 absorbs the one-time collectives barrier;
  - PE-critical tiles (x, weight streams, gT) live in the long-lived pool so
    they never pick up address anti-deps against the transient quant pool;
  - partial outputs, ReduceScatter and y are bf16 (host upcasts y to f32);
    per-block RS pipelines behind compute; the last block's RS is split in
    two halves to shorten the tail.
"""

import sys

sys.path.insert(0, "/opt/trn_rl_repo")

import numpy as np
import concourse.mybir as mybir
import concourse.tile as tile
import concourse.bass_isa as bass_isa
from concourse import bacc
from concourse.bass_utils import run_bass_kernel_spmd

F32 = mybir.dt.float32
F16 = mybir.dt.float16
BF16 = mybir.dt.bfloat16
FP8 = mybir.dt.float8e4
ALU = mybir.AluOpType
AX = mybir.AxisListType
ACTF = mybir.ActivationFunctionType

P = 128
TB = 512  # token-block width (matmul moving free dim)
CD = 1024  # phase A / wd processing column chunk
MAGIC = 12582912.0  # 1.5*2^23; add+sub rounds an f32 to nearest-even integer
EPS = 1e-5

# Full-problem config
FULL_T, FULL_H, FULL_I = 8192, 4096, 11008
N_CORES = 8

# Filled by kernel(); read by test.py
LAST_RESULTS = None


def shard_sizes(I_real, n_cores):
    i_s = -(-I_real // (P * n_cores)) * P  # per-core padded shard (mult of 128)
    return i_s, i_s // P


def build_bass(T=FULL_T, H=FULL_H, I_real=FULL_I, n_cores=N_CORES):
    assert T % TB == 0 and H % P == 0 and H % TB == 0 and TB % n_cores == 0
    HT = H // P  # contraction tiles for gate/up
    HB = H // TB  # down-phase output column blocks
    NB = T // TB  # token blocks
    TS = TB // P  # token sub-tiles per block (down-phase lhsT)
    i_s, IT = shard_sizes(I_real, n_cores)
    nreal = I_real * H  # real element count of each weight matrix
    rq = TB // n_cores  # ReduceScatter rows per core per block

    nc = bacc.Bacc("TRN2", target_bir_lowering=False, debug=False, num_devices=n_cores)
    xTb = nc.dram_tensor("xTb", [H, T], BF16, kind="ExternalInput")
    wgT = nc.dram_tensor("wgT", [H, i_s], F16, kind="ExternalInput")
    wuT = nc.dram_tensor("wuT", [H, i_s], F16, kind="ExternalInput")
    wdT = nc.dram_tensor("wdT", [i_s, H], F16, kind="ExternalInput")
    y = nc.dram_tensor("y", [NB, rq, H], BF16, kind="ExternalOutput")
    rg = [list(range(n_cores))]

    with tile.TileContext(nc) as tc:
        with tc.tile_pool(name="dram", bufs=1, space="DRAM") as dram:
            qu_d = dram.tile([IT, P, HT * P], FP8)  # up lhsT tiles, i-major
            qd_d = dram.tile([IT, P, H], FP8)  # down rhs tiles
            outb = [
                dram.tile([TB, H], BF16, name=f"outb{b}", tag=f"outb{b}")
                for b in range(NB)
            ]
            rsb = [
                dram.tile([rq, H], BF16, name=f"rsb{b}", tag=f"rsb{b}")
                for b in range(NB)
            ]
            # last block: two contiguous half-width buffers so its RS can be
            # split (collective inputs must be contiguous)
            outbL = [
                dram.tile([TB, H // 2], BF16, name=f"outbL{k}", tag=f"outbL{k}")
                for k in range(2)
            ]
            rsbL = [
                dram.tile([rq, H // 2], BF16, name=f"rsbL{k}", tag=f"rsbL{k}")
                for k in range(2)
            ]
            cc_in = [dram.tile([1, 1], F32, name=f"ccin{m}") for m in range(3)]
            cc_out = [
                dram.tile([1, 1], F32, addr_space="Shared", name=f"ccout{m}")
                for m in range(3)
            ]
            warm_in = dram.tile([1, 8], F32)
            warm_out = dram.tile([1, 8], F32, addr_space="Shared")

            with (
                tc.tile_pool(name="res", bufs=1) as rpool,
                tc.tile_pool(name="ps", bufs=8, space="PSUM") as pspool,
            ):
                # SBUF-resident quantized gate weights (ternary in fp8e4)
                qg_res = rpool.tile([P, HT, i_s], FP8)
                # manually double-buffered activation blocks and gate stage
                xb_res = [rpool.tile([P, HT, TB], BF16, name=f"xbr{j}") for j in range(2)]
                gT = [rpool.tile([P, IT, TB], BF16, name=f"gT{j}") for j in range(2)]
                rdenb = [rpool.tile([P, 1], F32, name=f"rdenb{m}") for m in range(3)]
                cb = rpool.tile([P, 1], F32)  # s_g*s_u*s_d, broadcast
                acc = rpool.tile([P, 4], F32)  # per-partition |w| sums
                sums = rpool.tile([1, 4], F32)
                gsum = [rpool.tile([1, 1], F32, name=f"gsum{m}") for m in range(3)]
                den = [rpool.tile([1, 1], F32, name=f"den{m}") for m in range(3)]
                rden = [rpool.tile([1, 1], F32, name=f"rden{m}") for m in range(3)]
                s3 = [rpool.tile([1, 1], F32, name=f"s3_{m}") for m in range(3)]
                cp01 = rpool.tile([1, 1], F32)
                cprod = rpool.tile([1, 1], F32)
                wsrc = rpool.tile([1, 8], F32)

                # ---------- warm-up collective: absorb the comms barrier ----------
                nc.vector.memset(wsrc, 0.0)
                nc.sync.dma_start(warm_in[:], wsrc[:])
                nc.gpsimd.collective_compute(
                    "AllReduce",
                    ALU.add,
                    ins=[warm_in[:]],
                    outs=[warm_out[:]],
                    replica_groups=rg,
                )

                def load_xb(b):
                    nc.sync.dma_start(
                        xb_res[b % 2][:],
                        xTb[:, b * TB : (b + 1) * TB].rearrange(
                            "(g p) f -> p g f", p=P
                        ),
                    )

                load_xb(0)
                load_xb(1)

                nc.vector.memset(acc, 0.0)
                rn = 1.0 / float(nreal)

                def qround(dst, src, m, pool, cols, nm):
                    # ACT does w*r+MAGIC (f32 add rounds to nearest-even int),
                    # DVE does -MAGIC & clamp low, then clamp high + fp8 cast.
                    t1 = pool.tile([P, cols], F32, tag=f"qt{cols}", name=f"qt_{nm}")
                    nc.scalar.activation(
                        t1, src, ACTF.Copy, bias=MAGIC, scale=rdenb[m][:, 0:1]
                    )
                    nc.vector.tensor_scalar(t1, t1, MAGIC, -1.0, ALU.subtract, ALU.max)
                    nc.vector.tensor_scalar(dst, t1, 1.0, None, ALU.min)

                with tc.tile_pool(name="quant", bufs=2) as qpool:
                    def phase_a(m, w, rows, cols, tag, engines):
                        # partial |w| sums in [P, CD] chunks on DVE and/or ACT
                        ci = 0
                        for r in range(rows):
                            for c0 in range(0, cols, CD):
                                cw = min(CD, cols - c0)
                                st = qpool.tile(
                                    [P, CD], F16, tag=tag, bufs=3,
                                    name=f"a{m}_{r}_{c0}",
                                )
                                nc.sync.dma_start(
                                    st[:, :cw], w[r * P : (r + 1) * P, c0 : c0 + cw]
                                )
                                part = qpool.tile(
                                    [P, 1], F32, tag="sp", bufs=4,
                                    name=f"sp{m}_{r}_{c0}",
                                )
                                eng = engines[ci % len(engines)]
                                ci += 1
                                if eng == "dve":
                                    nc.vector.tensor_reduce(
                                        part, st[:, :cw], axis=AX.X, op=ALU.add,
                                        apply_absolute_value=True,
                                    )
                                else:
                                    aout = qpool.tile(
                                        [P, CD], F16, tag="ao", bufs=2,
                                        name=f"ao{m}_{r}_{c0}",
                                    )
                                    nc.scalar.activation(
                                        aout[:, :cw], st[:, :cw], ACTF.Abs,
                                        accum_out=part,
                                    )
                                nc.vector.tensor_tensor(
                                    acc[:, m : m + 1], acc[:, m : m + 1], part,
                                    op=ALU.add,
                                )
                        # global scale via tiny AllReduce
                        allb = qpool.tile([P, 1], F32, tag="allb", bufs=2, name=f"al{m}")
                        nc.gpsimd.partition_all_reduce(
                            allb, acc[:, m : m + 1], P, bass_isa.ReduceOp.add
                        )
                        nc.vector.tensor_copy(sums[0:1, m : m + 1], allb[0:1, 0:1])
                        nc.sync.dma_start(cc_in[m][:], sums[0:1, m : m + 1])
                        nc.gpsimd.collective_compute(
                            "AllReduce", ALU.add, ins=[cc_in[m][:]],
                            outs=[cc_out[m][:]], replica_groups=rg,
                        )
                        nc.sync.dma_start(gsum[m][:], cc_out[m][:])
                        nc.vector.tensor_scalar(
                            den[m], gsum[m], rn, EPS, ALU.mult, ALU.add
                        )
                        nc.vector.reciprocal(rden[m], den[m])
                        nc.vector.tensor_scalar(s3[m], gsum[m], rn, None, ALU.mult)
                        nc.gpsimd.partition_broadcast(rdenb[m], rden[m])

                    # ---- gate: quantize into the SBUF-resident tile ----
                    phase_a(0, wgT, HT, i_s, "ag", ("dve", "act"))
                    for h in range(HT):
                        st = qpool.tile([P, i_s], F16, tag="qs", bufs=3, name=f"qsg{h}")
                        nc.sync.dma_start(st[:], wgT[h * P : (h + 1) * P, :])
                        qround(qg_res[:, h, :], st, 0, qpool, i_s, f"g{h}")

                    # ---- up/down scales after B(wg) so their chunk reads don't
                    # steal DMA bandwidth from the first-matmul critical path;
                    # the ACT-side reduces still overlap B(wg)'s DVE work ----
                    phase_a(1, wuT, HT, i_s, "ag", ("act",))
                    phase_a(2, wdT, IT, H, "ad", ("act",))

                    # ---- up: quantize to DRAM fp8, i-major ----
                    for h in range(HT):
                        st = qpool.tile([P, i_s], F16, tag="qs", bufs=3, name=f"qsu{h}")
                        nc.sync.dma_start(st[:], wuT[h * P : (h + 1) * P, :])
                        qb = qpool.tile([P, i_s], FP8, tag="qbu", bufs=2, name=f"qbu{h}")
                        qround(qb, st, 1, qpool, i_s, f"u{h}")
                        nc.sync.dma_start(
                            qu_d[:, :, h * P : (h + 1) * P].rearrange("i p f -> p i f"),
                            qb.rearrange("p (i f) -> p i f", i=IT),
                        )

                    # ---- down: quantize to DRAM fp8 in column chunks ----
                    for it in range(IT):
                        for c0 in range(0, H, CD):
                            st = qpool.tile(
                                [P, CD], F16, tag="ad", bufs=3, name=f"qsd{it}_{c0}"
                            )
                            nc.sync.dma_start(
                                st[:], wdT[it * P : (it + 1) * P, c0 : c0 + CD]
                            )
                            qb = qpool.tile(
                                [P, CD], FP8, tag="qbd", bufs=2, name=f"qbd{it}_{c0}"
                            )
                            qround(qb, st, 2, qpool, CD, f"d{it}_{c0}")
                            nc.sync.dma_start(qd_d[it, :, c0 : c0 + CD], qb[:])

                    nc.vector.tensor_tensor(cp01, s3[0], s3[1], op=ALU.mult)
                    nc.vector.tensor_tensor(cprod, cp01, s3[2], op=ALU.mult)
                    nc.gpsimd.partition_broadcast(cb, cprod)

                # ---------- Phase C: software-pipelined block loop ----------
                def gate_phase(b):
                    xb = xb_res[b % 2]
                    for i in range(IT):
                        pg = pspool.tile([P, TB], F32, tag="ps", name=f"pg{b}_{i}")
                        for h in range(HT):
                            nc.tensor.matmul(
                                pg,
                                lhsT=qg_res[:, h, i * P : (i + 1) * P],
                                rhs=xb[:, h, :],
                                start=(h == 0),
                                stop=(h == HT - 1),
                            )
                        nc.scalar.activation(gT[b % 2][:, i, :], pg, ACTF.Copy)

                with tc.tile_pool(name="main", bufs=2) as mpool:
                    gate_phase(0)
                    for b in range(NB):
                        # issue the first up-weight stream DMAs (one per buffer)
                        # ahead of the gate matmul burst so they beat the
                        # per-queue backlog without head-of-line blocking
                        qucs = {}
                        for i in range(3):
                            qucs[i] = rpool.tile(
                                [P, HT * P], FP8, tag="quc", bufs=3, name=f"quc{b}_{i}"
                            )
                            nc.sync.dma_start(qucs[i][:], qu_d[i])
                        if b + 1 < NB:
                            gate_phase(b + 1)
                        xb = xb_res[b % 2]
                        interT = mpool.tile(
                            [P, IT, TB], BF16, tag="inter", bufs=1, name=f"int{b}"
                        )
                        for i in range(IT):
                            if i in qucs:
                                quc = qucs[i]
                            else:
                                quc = rpool.tile(
                                    [P, HT * P], FP8, tag="quc", bufs=3,
                                    name=f"quc{b}_{i}",
                                )
                                nc.sync.dma_start(quc[:], qu_d[i])
                            pu = pspool.tile([P, TB], F32, tag="ps", name=f"pu{b}_{i}")
                            for h in range(HT):
                                nc.tensor.matmul(
                                    pu,
                                    lhsT=quc[:, h * P : (h + 1) * P],
                                    rhs=xb[:, h, :],
                                    start=(h == 0),
                                    stop=(h == HT - 1),
                                )
                            nc.vector.tensor_tensor(
                                interT[:, i, :], pu, gT[b % 2][:, i, :], op=ALU.mult
                            )
                        if b + 2 < NB:
                            load_xb(b + 2)
                        for hb in range(HB):
                            qdc = rpool.tile(
                                [P, IT, TB], FP8, tag="qdc", bufs=3, name=f"qdc{b}_{hb}"
                            )
                            nc.sync.dma_start(
                                qdc[:],
                                qd_d[:, :, hb * TB : (hb + 1) * TB].rearrange(
                                    "i p f -> p i f"
                                ),
                            )
                            pos = [
                                pspool.tile(
                                    [P, TB], F32, tag="ps", name=f"po{b}_{hb}_{t}"
                                )
                                for t in range(TS)
                            ]
                            for i in range(IT):
                                for ts in range(TS):
                                    nc.tensor.matmul(
                                        pos[ts],
                                        lhsT=interT[:, i, ts * P : (ts + 1) * P],
                                        rhs=qdc[:, i, :],
                                        start=(i == 0),
                                        stop=(i == IT - 1),
                                    )
                            ob = mpool.tile(
                                [P, TS, TB], BF16, tag="ob", bufs=2, name=f"ob{b}_{hb}"
                            )
                            for ts in range(TS):
                                nc.vector.tensor_scalar(
                                    ob[:, ts, :], pos[ts], cb[:, 0:1], None, ALU.mult
                                )
                            if b == NB - 1:
                                hpb = HB // 2  # hb blocks per half
                                nc.sync.dma_start(
                                    outbL[hb // hpb][
                                        :, (hb % hpb) * TB : (hb % hpb + 1) * TB
                                    ].rearrange("(g p) f -> p g f", p=P),
                                    ob[:],
                                )
                                # fire each half's RS as soon as it is complete
                                if hb % hpb == hpb - 1:
                                    k = hb // hpb
                                    h2 = H // 2
                                    nc.gpsimd.collective_compute(
                                        "ReduceScatter",
                                        ALU.add,
                                        ins=[outbL[k][:]],
                                        outs=[rsbL[k][:]],
                                        replica_groups=rg,
                                    )
                                    nc.sync.dma_start(
                                        y[b, :, k * h2 : (k + 1) * h2], rsbL[k][:]
                                    )
                            else:
                                nc.sync.dma_start(
                                    outb[b][:, hb * TB : (hb + 1) * TB].rearrange(
                                        "(g p) f -> p g f", p=P
                                    ),
                                    ob[:],
                                )
                        if b != NB - 1:
                            nc.gpsimd.collective_compute(
                                "ReduceScatter",
                                ALU.add,
                                ins=[outb[b][:]],
                                outs=[rsb[b][:]],
                                replica_groups=rg,
                            )
                            nc.sync.dma_start(y[b], rsb[b][:])
    nc.compile()
    return nc


_NC_CACHE = {}


def _get_nc(T, H, I_real, n_cores):
    key = (T, H, I_real, n_cores)
    if key not in _NC_CACHE:
        _NC_CACHE[key] = build_bass(T, H, I_real, n_cores)
    return _NC_CACHE[key]


def shard_inputs(hidden_states, w_gate, w_up, w_down, n_cores=N_CORES):
    """Host prep: flatten/transpose/zero-pad/slice; activations cast to bf16,
    weights to f16 (scale + ternarization still computed on device)."""
    B, S, H = hidden_states.shape
    T = B * S
    I_real = w_gate.shape[0]
    i_s, _ = shard_sizes(I_real, n_cores)
    Ip = i_s * n_cores
    bf16 = mybir.dt.np(BF16)

    xTb = np.ascontiguousarray(
        hidden_states.reshape(T, H).T.astype(np.float32, copy=False)
    ).astype(bf16)
    wgT = np.zeros((H, Ip), np.float16)
    wgT[:, :I_real] = w_gate.T
    wuT = np.zeros((H, Ip), np.float16)
    wuT[:, :I_real] = w_up.T
    wdT = np.zeros((Ip, H), np.float16)
    wdT[:I_real, :] = w_down.T

    in_maps = []
    for c in range(n_cores):
        in_maps.append(
            {
                "xTb": xTb,
                "wgT": np.ascontiguousarray(wgT[:, c * i_s : (c + 1) * i_s]),
                "wuT": np.ascontiguousarray(wuT[:, c * i_s : (c + 1) * i_s]),
                "wdT": np.ascontiguousarray(wdT[c * i_s : (c + 1) * i_s, :]),
            }
        )
    return in_maps, (B, S, H, T)


def kernel(hidden_states, w_gate, w_up, w_down, _trace=False):
    global LAST_RESULTS
    n_cores = N_CORES
    in_maps, (B, S, H, T) = shard_inputs(hidden_states, w_gate, w_up, w_down, n_cores)
    I_real = w_gate.shape[0]
    nc = _get_nc(T, H, I_real, n_cores)
    res = run_bass_kernel_spmd(
        nc, in_maps, core_ids=list(range(n_cores)), trace=_trace
    )
    LAST_RESULTS = res

    NB = T // TB
    rq = TB // n_cores
    out = np.empty((T, H), np.float32)
    for c in range(n_cores):
        yc = res.results[c]["y"]  # [NB, rq, H] bf16
        for b in range(NB):
            out[b * TB + c * rq : b * TB + (c + 1) * rq] = yc[b].astype(np.float32)
    return out.reshape(B, S, H)


# revision 19
# speedup vs baseline: 1.0073x; 1.0073x over previous
"""Trainium2 Bass kernel for nn_LlamaMLP (BitLinear-style ternary-quantized MLP).

Reference computation (all f32):
    s_m   = mean(|w_m|)                            (global scalar per weight)
    q_m   = round(clip(w_m / (s_m + eps), -1, 1))  (ternary)
    gate  = x @ (q_g * s_g).T ; up = x @ (q_u * s_u).T
    out   = (gate * up) @ (q_d * s_d).T
        == (s_g*s_u*s_d) * ((x @ q_g.T) * (x @ q_u.T)) @ q_d.T

Strategy: tensor-parallel over the intermediate dim I. Each core gets f16
weight shards and quantizes them on device to ternary fp8e4 (exact). The gate
weights live SBUF-resident; up/down weights go to DRAM fp8 and stream per
block (~22us/block, hidden). Matmuls run bf16 activations x fp8 ternary
weights (mixed dtype, bf16 rate) with f32 PSUM accumulation.

Critical-path engineering:
  - per-matrix pipeline A(w) -> AllReduce(scale) -> B(quantize) ordered gate,
    up, down. A(wg) splits its |w| reduce across DVE and ACT; A(wu)/A(wd)
    run entirely on ACT (activation Abs + accum_out) so they never queue
    behind B quantize work on DVE. Block 0's gate matmuls start ~140us in.
  - the block loop is software-pipelined as [gate(b+1), up(b), down(b)] so
    the PE always has gate work in hand even if up/down weight streams lag;
  - a dummy AllReduce at t=0 absorbs the one-time collectives barrier;
  - PE-critical tiles (x, weight streams, gT) live in the long-lived pool so
    they never pick up address anti-deps against the transient quant pool;
  - partial outputs, ReduceScatter and y are bf16 (host upcasts y to f32);
    per-block RS pipelines behind compute; the last block's RS is split in
    two halves to shorten the tail.
"""

import sys

sys.path.insert(0, "/opt/trn_rl_repo")

import numpy as np
import concourse.mybir as mybir
import concourse.tile as tile
import concourse.bass_isa as bass_isa
from concourse import bacc
from concourse.bass_utils import run_bass_kernel_spmd

F32 = mybir.dt.float32
F16 = mybir.dt.float16
BF16 = mybir.dt.bfloat16
FP8 = mybir.dt.float8e4
ALU = mybir.AluOpType
AX = mybir.AxisListType
ACTF = mybir.ActivationFunctionType

P = 128
TB = 512  # token-block width (matmul moving free dim)
CD = 1024  # phase A / wd processing column chunk
MAGIC = 12582912.0  # 1.5*2^23; add+sub rounds an f32 to nearest-even integer
EPS = 1e-5

# Full-problem config
FULL_T, FULL_H, FULL_I = 8192, 4096, 11008
N_CORES = 8

# Filled by kernel(); read by test.py
LAST_RESULTS = None


def shard_sizes(I_real, n_cores):
    i_s = -(-I_real // (P * n_cores)) * P  # per-core padded shard (mult of 128)
    return i_s, i_s // P


def build_bass(T=FULL_T, H=FULL_H, I_real=FULL_I, n_cores=N_CORES):
    assert T % TB == 0 and H % P == 0 and H % TB == 0 and TB % n_cores == 0
    HT = H // P  # contraction tiles for gate/up
    HB = H // TB  # down-phase output column blocks
    NB = T // TB  # token blocks
    TS = TB // P  # token sub-tiles per block (down-phase lhsT)
    i_s, IT = shard_sizes(I_real, n_cores)
    nreal = I_real * H  # real element count of each weight matrix
    rq = TB // n_cores  # ReduceScatter rows per core per block

    nc = bacc.Bacc("TRN2", target_bir_lowering=False, debug=False, num_devices=n_cores)
    xTb = nc.dram_tensor("xTb", [H, T], BF16, kind="ExternalInput")
    wgT = nc.dram_tensor("wgT", [H, i_s], F16, kind="ExternalInput")
    # wuT is host-prearranged into the i-major up-lhsT layout:
    #   wuT[i*P + p, h*P + f] = w_up.T[h*P + p, i*P + f]
    wuT = nc.dram_tensor("wuT", [i_s, HT * P], F16, kind="ExternalInput")
    wdT = nc.dram_tensor("wdT", [i_s, H], F16, kind="ExternalInput")
    y = nc.dram_tensor("y", [NB, rq, H], BF16, kind="ExternalOutput")
    rg = [list(range(n_cores))]

    with tile.TileContext(nc) as tc:
        with tc.tile_pool(name="dram", bufs=1, space="DRAM") as dram:
            qu_d = dram.tile([IT, P, HT * P], FP8)  # up lhsT tiles, i-major
            qd_d = dram.tile([IT, P, H], FP8)  # down rhs tiles
            outb = [
                dram.tile([TB, H], BF16, name=f"outb{b}", tag=f"outb{b}")
                for b in range(NB)
            ]
            rsb = [
                dram.tile([rq, H], BF16, name=f"rsb{b}", tag=f"rsb{b}")
                for b in range(NB)
            ]
            # last block: two contiguous half-width buffers so its RS can be
            # split (collective inputs must be contiguous)
            outbL = [
                dram.tile([TB, H // 2], BF16, name=f"outbL{k}", tag=f"outbL{k}")
                for k in range(2)
            ]
            rsbL = [
                dram.tile([rq, H // 2], BF16, name=f"rsbL{k}", tag=f"rsbL{k}")
                for k in range(2)
            ]
            cc_in = [dram.tile([1, 1], F32, name=f"ccin{m}") for m in range(3)]
            cc_out = [
                dram.tile([1, 1], F32, addr_space="Shared", name=f"ccout{m}")
                for m in range(3)
            ]
            warm_in = dram.tile([1, 8], F32)
            warm_out = dram.tile([1, 8], F32, addr_space="Shared")

            with (
                tc.tile_pool(name="res", bufs=1) as rpool,
                tc.tile_pool(name="ps", bufs=8, space="PSUM") as pspool,
            ):
                # SBUF-resident quantized gate weights (ternary in fp8e4)
                qg_res = rpool.tile([P, HT, i_s], FP8)
                # manually double-buffered activation blocks and gate stage
                xb_res = [rpool.tile([P, HT, TB], BF16, name=f"xbr{j}") for j in range(2)]
                gT = [rpool.tile([P, IT, TB], BF16, name=f"gT{j}") for j in range(2)]
                rdenb = [rpool.tile([P, 1], F32, name=f"rdenb{m}") for m in range(3)]
                cb = rpool.tile([P, 1], F32)  # s_g*s_u*s_d, broadcast
                acc = rpool.tile([P, 4], F32)  # per-partition |w| sums
                sums = rpool.tile([1, 4], F32)
                gsum = [rpool.tile([1, 1], F32, name=f"gsum{m}") for m in range(3)]
                den = [rpool.tile([1, 1], F32, name=f"den{m}") for m in range(3)]
                rden = [rpool.tile([1, 1], F32, name=f"rden{m}") for m in range(3)]
                s3 = [rpool.tile([1, 1], F32, name=f"s3_{m}") for m in range(3)]
                cp01 = rpool.tile([1, 1], F32)
                cprod = rpool.tile([1, 1], F32)
                wsrc = rpool.tile([1, 8], F32)

                # ---------- warm-up collective: absorb the comms barrier ----------
                nc.vector.memset(wsrc, 0.0)
                nc.sync.dma_start(warm_in[:], wsrc[:])
                nc.gpsimd.collective_compute(
                    "AllReduce",
                    ALU.add,
                    ins=[warm_in[:]],
                    outs=[warm_out[:]],
                    replica_groups=rg,
                )

                def load_xb(b):
                    nc.sync.dma_start(
                        xb_res[b % 2][:],
                        xTb[:, b * TB : (b + 1) * TB].rearrange(
                            "(g p) f -> p g f", p=P
                        ),
                    )

                load_xb(0)
                load_xb(1)

                nc.vector.memset(acc, 0.0)
                rn = 1.0 / float(nreal)

                def qround(dst, src, m, pool, cols, nm):
                    # ACT does w*r+MAGIC (f32 add rounds to nearest-even int),
                    # DVE does -MAGIC & clamp low, then clamp high + fp8 cast.
                    t1 = pool.tile([P, cols], F32, tag=f"qt{cols}", name=f"qt_{nm}")
                    nc.scalar.activation(
                        t1, src, ACTF.Copy, bias=MAGIC, scale=rdenb[m][:, 0:1]
                    )
                    nc.vector.tensor_scalar(t1, t1, MAGIC, -1.0, ALU.subtract, ALU.max)
                    nc.vector.tensor_scalar(dst, t1, 1.0, None, ALU.min)

                with tc.tile_pool(name="quant", bufs=2) as qpool:
                    def phase_a(m, w, rows, cols, tag, engines):
                        # partial |w| sums in [P, CD] chunks on DVE and/or ACT
                        ci = 0
                        for r in range(rows):
                            for c0 in range(0, cols, CD):
                                cw = min(CD, cols - c0)
                                st = qpool.tile(
                                    [P, CD], F16, tag=tag, bufs=3,
                                    name=f"a{m}_{r}_{c0}",
                                )
                                nc.sync.dma_start(
                                    st[:, :cw], w[r * P : (r + 1) * P, c0 : c0 + cw]
                                )
                                part = qpool.tile(
                                    [P, 1], F32, tag="sp", bufs=4,
                                    name=f"sp{m}_{r}_{c0}",
                                )
                                eng = engines[ci % len(engines)]
                                ci += 1
                                if eng == "dve":
                                    nc.vector.tensor_reduce(
                                        part, st[:, :cw], axis=AX.X, op=ALU.add,
                                        apply_absolute_value=True,
                                    )
                                else:
                                    aout = qpool.tile(
                                        [P, CD], F16, tag="ao", bufs=2,
                                        name=f"ao{m}_{r}_{c0}",
                                    )
                                    nc.scalar.activation(
                                        aout[:, :cw], st[:, :cw], ACTF.Abs,
                                        accum_out=part,
                                    )
                                nc.vector.tensor_tensor(
                                    acc[:, m : m + 1], acc[:, m : m + 1], part,
                                    op=ALU.add,
                                )
                        # global scale via tiny AllReduce
                        allb = qpool.tile([P, 1], F32, tag="allb", bufs=2, name=f"al{m}")
                        nc.gpsimd.partition_all_reduce(
                            allb, acc[:, m : m + 1], P, bass_isa.ReduceOp.add
                        )
                        nc.vector.tensor_copy(sums[0:1, m : m + 1], allb[0:1, 0:1])
                        nc.sync.dma_start(cc_in[m][:], sums[0:1, m : m + 1])
                        nc.gpsimd.collective_compute(
                            "AllReduce", ALU.add, ins=[cc_in[m][:]],
                            outs=[cc_out[m][:]], replica_groups=rg,
                        )
                        nc.sync.dma_start(gsum[m][:], cc_out[m][:])
                        nc.vector.tensor_scalar(
                            den[m], gsum[m], rn, EPS, ALU.mult, ALU.add
                        )
                        nc.vector.reciprocal(rden[m], den[m])
                        nc.vector.tensor_scalar(s3[m], gsum[m], rn, None, ALU.mult)
                        nc.gpsimd.partition_broadcast(rdenb[m], rden[m])

                    # ---- gate: quantize into the SBUF-resident tile ----
                    phase_a(0, wgT, HT, i_s, "ag", ("dve", "act"))
                    for h in range(HT):
                        st = qpool.tile([P, i_s], F16, tag="qs", bufs=3, name=f"qsg{h}")
                        nc.sync.dma_start(st[:], wgT[h * P : (h + 1) * P, :])
                        qround(qg_res[:, h, :], st, 0, qpool, i_s, f"g{h}")

                    # ---- up/down scales after B(wg) so their chunk reads don't
                    # steal DMA bandwidth from the first-matmul critical path;
                    # the ACT-side reduces still overlap B(wg)'s DVE work ----
                    phase_a(1, wuT, IT, HT * P, "ad", ("act",))
                    phase_a(2, wdT, IT, H, "ad", ("act",))

                    # ---- up: quantize to DRAM fp8. wuT arrives host-prearranged
                    # in the i-major lhsT layout, so reads AND writes are fully
                    # contiguous (the old h-major layout forced 128B-run strided
                    # writes that delayed qu_d by ~250us) ----
                    for it in range(IT):
                        for c0 in range(0, HT * P, CD):
                            st = qpool.tile(
                                [P, CD], F16, tag="ad", bufs=3, name=f"qsu{it}_{c0}"
                            )
                            nc.sync.dma_start(
                                st[:], wuT[it * P : (it + 1) * P, c0 : c0 + CD]
                            )
                            qb = qpool.tile(
                                [P, CD], FP8, tag="qbd", bufs=2, name=f"qbu{it}_{c0}"
                            )
                            qround(qb, st, 1, qpool, CD, f"u{it}_{c0}")
                            nc.sync.dma_start(qu_d[it, :, c0 : c0 + CD], qb[:])

                    # ---- down: quantize to DRAM fp8 in column chunks ----
                    for it in range(IT):
                        for c0 in range(0, H, CD):
                            st = qpool.tile(
                                [P, CD], F16, tag="ad", bufs=3, name=f"qsd{it}_{c0}"
                            )
                            nc.sync.dma_start(
                                st[:], wdT[it * P : (it + 1) * P, c0 : c0 + CD]
                            )
                            qb = qpool.tile(
                                [P, CD], FP8, tag="qbd", bufs=2, name=f"qbd{it}_{c0}"
                            )
                            qround(qb, st, 2, qpool, CD, f"d{it}_{c0}")
                            nc.sync.dma_start(qd_d[it, :, c0 : c0 + CD], qb[:])

                    nc.vector.tensor_tensor(cp01, s3[0], s3[1], op=ALU.mult)
                    nc.vector.tensor_tensor(cprod, cp01, s3[2], op=ALU.mult)
                    nc.gpsimd.partition_broadcast(cb, cprod)

                # ---------- Phase C: software-pipelined block loop ----------
                def gate_phase(b):
                    xb = xb_res[b % 2]
                    for i in range(IT):
                        pg = pspool.tile([P, TB], F32, tag="ps", name=f"pg{b}_{i}")
                        for h in range(HT):
                            nc.tensor.matmul(
                                pg,
                                lhsT=qg_res[:, h, i * P : (i + 1) * P],
                                rhs=xb[:, h, :],
                                start=(h == 0),
                                stop=(h == HT - 1),
                            )
                        nc.scalar.activation(gT[b % 2][:, i, :], pg, ACTF.Copy)

                with tc.tile_pool(name="main", bufs=2) as mpool:
                    gate_phase(0)
                    for b in range(NB):
                        # issue the first up-weight stream DMAs (one per buffer)
                        # ahead of the gate matmul burst so they beat the
                        # per-queue backlog without head-of-line blocking
                        qucs = {}
                        for i in range(3):
                            qucs[i] = rpool.tile(
                                [P, HT * P], FP8, tag="quc", bufs=3, name=f"quc{b}_{i}"
                            )
                            nc.sync.dma_start(qucs[i][:], qu_d[i])
                        if b + 1 < NB:
                            gate_phase(b + 1)
                        xb = xb_res[b % 2]
                        interT = mpool.tile(
                            [P, IT, TB], BF16, tag="inter", bufs=1, name=f"int{b}"
                        )
                        for i in range(IT):
                            if i in qucs:
                                quc = qucs[i]
                            else:
                                quc = rpool.tile(
                                    [P, HT * P], FP8, tag="quc", bufs=3,
                                    name=f"quc{b}_{i}",
                                )
                                nc.sync.dma_start(quc[:], qu_d[i])
                            pu = pspool.tile([P, TB], F32, tag="ps", name=f"pu{b}_{i}")
                            for h in range(HT):
                                nc.tensor.matmul(
                                    pu,
                                    lhsT=quc[:, h * P : (h + 1) * P],
                                    rhs=xb[:, h, :],
                                    start=(h == 0),
                                    stop=(h == HT - 1),
                                )
                            nc.vector.tensor_tensor(
                                interT[:, i, :], pu, gT[b % 2][:, i, :], op=ALU.mult
                            )
                        if b + 2 < NB:
                            load_xb(b + 2)
                        for hb in range(HB):
                            qdc = rpool.tile(
                                [P, IT, TB], FP8, tag="qdc", bufs=3, name=f"qdc{b}_{hb}"
                            )
                            nc.sync.dma_start(
                                qdc[:],
                                qd_d[:, :, hb * TB : (hb + 1) * TB].rearrange(
                                    "i p f -> p i f"
                                ),
                            )
                            pos = [
                                pspool.tile(
                                    [P, TB], F32, tag="ps", name=f"po{b}_{hb}_{t}"
                                )
                                for t in range(TS)
                            ]
                            for i in range(IT):
                                for ts in range(TS):
                                    nc.tensor.matmul(
                                        pos[ts],
                                        lhsT=interT[:, i, ts * P : (ts + 1) * P],
                                        rhs=qdc[:, i, :],
                                        start=(i == 0),
                                        stop=(i == IT - 1),
                                    )
                            ob = mpool.tile(
                                [P, TS, TB], BF16, tag="ob", bufs=2, name=f"ob{b}_{hb}"
                            )
                            for ts in range(TS):
                                nc.vector.tensor_scalar(
                                    ob[:, ts, :], pos[ts], cb[:, 0:1], None, ALU.mult
                                )
                            if b == NB - 1:
                                hpb = HB // 2  # hb blocks per half
                                nc.sync.dma_start(
                                    outbL[hb // hpb][
                                        :, (hb % hpb) * TB : (hb % hpb + 1) * TB
                                    ].rearrange("(g p) f -> p g f", p=P),
                                    ob[:],
                                )
                                # fire each half's RS as soon as it is complete
                                if hb % hpb == hpb - 1:
                                    k = hb // hpb
                                    h2 = H // 2
                                    nc.gpsimd.collective_compute(
                                        "ReduceScatter",
                                        ALU.add,
                                        ins=[outbL[k][:]],
                                        outs=[rsbL[k][:]],
                                        replica_groups=rg,
                                    )
                                    nc.sync.dma_start(
                                        y[b, :, k * h2 : (k + 1) * h2], rsbL[k][:]
                                    )
                            else:
                                nc.sync.dma_start(
                                    outb[b][:, hb * TB : (hb + 1) * TB].rearrange(
                                        "(g p) f -> p g f", p=P
                                    ),
                                    ob[:],
                                )
                        if b != NB - 1:
                            nc.gpsimd.collective_compute(
                                "ReduceScatter",
                                ALU.add,
                                ins=[outb[b][:]],
                                outs=[rsb[b][:]],
                                replica_groups=rg,
                            )
                            nc.sync.dma_start(y[b], rsb[b][:])
    nc.compile()
    return nc


_NC_CACHE = {}


def _get_nc(T, H, I_real, n_cores):
    key = (T, H, I_real, n_cores)
    if key not in _NC_CACHE:
        _NC_CACHE[key] = build_bass(T, H, I_real, n_cores)
    return _NC_CACHE[key]


def shard_inputs(hidden_states, w_gate, w_up, w_down, n_cores=N_CORES):
    """Host prep: flatten/transpose/zero-pad/slice; activations cast to bf16,
    weights to f16 (scale + ternarization still computed on device)."""
    B, S, H = hidden_states.shape
    T = B * S
    I_real = w_gate.shape[0]
    i_s, _ = shard_sizes(I_real, n_cores)
    Ip = i_s * n_cores
    bf16 = mybir.dt.np(BF16)

    xTb = np.ascontiguousarray(
        hidden_states.reshape(T, H).T.astype(np.float32, copy=False)
    ).astype(bf16)
    wgT = np.zeros((H, Ip), np.float16)
    wgT[:, :I_real] = w_gate.T
    wuT = np.zeros((H, Ip), np.float16)
    wuT[:, :I_real] = w_up.T
    wdT = np.zeros((Ip, H), np.float16)
    wdT[:I_real, :] = w_down.T

    HT, IT = H // P, i_s // P
    in_maps = []
    for c in range(n_cores):
        wu_c = wuT[:, c * i_s : (c + 1) * i_s]  # [H, i_s]
        # i-major up-lhsT layout: [i, p, h, f] <- [h*P+p, i*P+f]
        wu_dev = np.ascontiguousarray(
            wu_c.reshape(HT, P, IT, P).transpose(2, 1, 0, 3).reshape(i_s, H)
        )
        in_maps.append(
            {
                "xTb": xTb,
                "wgT": np.ascontiguousarray(wgT[:, c * i_s : (c + 1) * i_s]),
                "wuT": wu_dev,
                "wdT": np.ascontiguousarray(wdT[c * i_s : (c + 1) * i_s, :]),
            }
        )
    return in_maps, (B, S, H, T)


def kernel(hidden_states, w_gate, w_up, w_down, _trace=False):
    global LAST_RESULTS
    n_cores = N_CORES
    in_maps, (B, S, H, T) = shard_inputs(hidden_states, w_gate, w_up, w_down, n_cores)
    I_real = w_gate.shape[0]
    nc = _get_nc(T, H, I_real, n_cores)
    res = run_bass_kernel_spmd(
        nc, in_maps, core_ids=list(range(n_cores)), trace=_trace
    )
    LAST_RESULTS = res

    NB = T // TB
    rq = TB // n_cores
    out = np.empty((T, H), np.float32)
    for c in range(n_cores):
        yc = res.results[c]["y"]  # [NB, rq, H] bf16
        for b in range(NB):
            out[b * TB + c * rq : b * TB + (c + 1) * rq] = yc[b].astype(np.float32)
    return out.reshape(B, S, H)


# revision 21
# speedup vs baseline: 1.0210x; 1.0136x over previous
"""Trainium2 Bass kernel for nn_LlamaMLP (BitLinear-style ternary-quantized MLP).

Reference computation (all f32):
    s_m   = mean(|w_m|)                            (global scalar per weight)
    q_m   = round(clip(w_m / (s_m + eps), -1, 1))  (ternary)
    gate  = x @ (q_g * s_g).T ; up = x @ (q_u * s_u).T
    out   = (gate * up) @ (q_d * s_d).T
        == (s_g*s_u*s_d) * ((x @ q_g.T) * (x @ q_u.T)) @ q_d.T

Strategy: tensor-parallel over the intermediate dim I. Each core gets f16
weight shards and quantizes them on device to ternary fp8e4 (exact). The gate
weights live SBUF-resident; up/down weights go to DRAM fp8 and stream per
block (~22us/block, hidden). Matmuls run bf16 activations x fp8 ternary
weights (mixed dtype, bf16 rate) with f32 PSUM accumulation.

Critical-path engineering:
  - per-matrix pipeline A(w) -> AllReduce(scale) -> B(quantize) ordered gate,
    up, down. A(wg) splits its |w| reduce across DVE and ACT; A(wu)/A(wd)
    run entirely on ACT (activation Abs + accum_out) so they never queue
    behind B quantize work on DVE. Block 0's gate matmuls start ~140us in.
  - the block loop is software-pipelined as [gate(b+1), up(b), down(b)] so
    the PE always has gate work in hand even if up/down weight streams lag;
  - a dummy AllReduce at t=0 absorbs the one-time collectives barrier;
  - PE-critical tiles (x, weight streams, gT) live in the long-lived pool so
    they never pick up address anti-deps against the transient quant pool;
  - partial outputs, ReduceScatter and y are bf16 (host upcasts y to f32);
    per-block RS pipelines behind compute; the last block's RS is split in
    two halves to shorten the tail.
"""

import sys

sys.path.insert(0, "/opt/trn_rl_repo")

import numpy as np
import concourse.mybir as mybir
import concourse.tile as tile
import concourse.bass_isa as bass_isa
from concourse import bacc
from concourse.bass_utils import run_bass_kernel_spmd

F32 = mybir.dt.float32
F16 = mybir.dt.float16
BF16 = mybir.dt.bfloat16
FP8 = mybir.dt.float8e4
ALU = mybir.AluOpType
AX = mybir.AxisListType
ACTF = mybir.ActivationFunctionType

P = 128
TB = 512  # token-block width (matmul moving free dim)
CD = 1024  # phase A / wd processing column chunk
MAGIC = 12582912.0  # 1.5*2^23; add+sub rounds an f32 to nearest-even integer
EPS = 1e-5

# Full-problem config
FULL_T, FULL_H, FULL_I = 8192, 4096, 11008
N_CORES = 8

# Filled by kernel(); read by test.py
LAST_RESULTS = None


def shard_sizes(I_real, n_cores):
    i_s = -(-I_real // (P * n_cores)) * P  # per-core padded shard (mult of 128)
    return i_s, i_s // P


def build_bass(T=FULL_T, H=FULL_H, I_real=FULL_I, n_cores=N_CORES):
    assert T % TB == 0 and H % P == 0 and H % TB == 0 and TB % n_cores == 0
    HT = H // P  # contraction tiles for gate/up
    HB = H // TB  # down-phase output column blocks
    NB = T // TB  # token blocks
    TS = TB // P  # token sub-tiles per block (down-phase lhsT)
    i_s, IT = shard_sizes(I_real, n_cores)
    nreal = I_real * H  # real element count of each weight matrix
    rq = TB // n_cores  # ReduceScatter rows per core per block

    nc = bacc.Bacc("TRN2", target_bir_lowering=False, debug=False, num_devices=n_cores)
    xTb = nc.dram_tensor("xTb", [H, T], BF16, kind="ExternalInput")
    wgT = nc.dram_tensor("wgT", [H, i_s], F16, kind="ExternalInput")
    # wuT is host-prearranged into the i-major up-lhsT layout:
    #   wuT[i*P + p, h*P + f] = w_up.T[h*P + p, i*P + f]
    wuT = nc.dram_tensor("wuT", [i_s, HT * P], F16, kind="ExternalInput")
    wdT = nc.dram_tensor("wdT", [i_s, H], F16, kind="ExternalInput")
    y = nc.dram_tensor("y", [NB, rq, H], BF16, kind="ExternalOutput")
    rg = [list(range(n_cores))]

    with tile.TileContext(nc) as tc:
        with tc.tile_pool(name="dram", bufs=1, space="DRAM") as dram:
            qu_d = dram.tile([IT, P, HT * P], FP8)  # up lhsT tiles, i-major
            qd_d = dram.tile([IT, P, H], FP8)  # down rhs tiles
            outb = [
                dram.tile([TB, H], BF16, name=f"outb{b}", tag=f"outb{b}")
                for b in range(NB)
            ]
            rsb = [
                dram.tile([rq, H], BF16, name=f"rsb{b}", tag=f"rsb{b}")
                for b in range(NB)
            ]
            # last block: two contiguous half-width buffers so its RS can be
            # split (collective inputs must be contiguous)
            outbL = [
                dram.tile([TB, H // 2], BF16, name=f"outbL{k}", tag=f"outbL{k}")
                for k in range(2)
            ]
            rsbL = [
                dram.tile([rq, H // 2], BF16, name=f"rsbL{k}", tag=f"rsbL{k}")
                for k in range(2)
            ]
            cc_in = [dram.tile([1, 1], F32, name=f"ccin{m}") for m in range(3)]
            cc_out = [
                dram.tile([1, 1], F32, addr_space="Shared", name=f"ccout{m}")
                for m in range(3)
            ]
            warm_in = dram.tile([1, 8], F32)
            warm_out = dram.tile([1, 8], F32, addr_space="Shared")

            with (
                tc.tile_pool(name="res", bufs=1) as rpool,
                tc.tile_pool(name="ps", bufs=8, space="PSUM") as pspool,
            ):
                # SBUF-resident quantized gate weights (ternary in fp8e4)
                qg_res = rpool.tile([P, HT, i_s], FP8)
                # manually double-buffered activation blocks and gate stage
                xb_res = [rpool.tile([P, HT, TB], BF16, name=f"xbr{j}") for j in range(2)]
                gT = [rpool.tile([P, IT, TB], BF16, name=f"gT{j}") for j in range(2)]
                rdenb = [rpool.tile([P, 1], F32, name=f"rdenb{m}") for m in range(3)]
                cb = rpool.tile([P, 1], F32)  # s_g*s_u*s_d, broadcast
                acc = rpool.tile([P, 4], F32)  # per-partition |w| sums
                sums = rpool.tile([1, 4], F32)
                gsum = [rpool.tile([1, 1], F32, name=f"gsum{m}") for m in range(3)]
                den = [rpool.tile([1, 1], F32, name=f"den{m}") for m in range(3)]
                rden = [rpool.tile([1, 1], F32, name=f"rden{m}") for m in range(3)]
                s3 = [rpool.tile([1, 1], F32, name=f"s3_{m}") for m in range(3)]
                cp01 = rpool.tile([1, 1], F32)
                cprod = rpool.tile([1, 1], F32)
                wsrc = rpool.tile([1, 8], F32)

                # ---------- warm-up collective: absorb the comms barrier ----------
                nc.vector.memset(wsrc, 0.0)
                nc.sync.dma_start(warm_in[:], wsrc[:])
                nc.gpsimd.collective_compute(
                    "AllReduce",
                    ALU.add,
                    ins=[warm_in[:]],
                    outs=[warm_out[:]],
                    replica_groups=rg,
                )

                def load_xb(b):
                    nc.sync.dma_start(
                        xb_res[b % 2][:],
                        xTb[:, b * TB : (b + 1) * TB].rearrange(
                            "(g p) f -> p g f", p=P
                        ),
                    )

                # pin the weight-stream tag addresses NOW, before the quant pool
                # opens: otherwise their first (lazy) allocations land in freed
                # quant-pool space and the phase-C stream DMAs inherit an
                # anti-dependency on the entire quantization DVE backlog
                for j in range(3):
                    rpool.tile([P, HT * P], FP8, tag="quc", bufs=3, name=f"qucpin{j}")
                    rpool.tile([P, IT, TB], FP8, tag="qdc", bufs=3, name=f"qdcpin{j}")

                nc.vector.memset(acc, 0.0)
                rn = 1.0 / float(nreal)

                def qround(dst, src, m, pool, cols, nm):
                    # ACT does w*r+MAGIC (f32 add rounds to nearest-even int),
                    # DVE does -MAGIC & clamp low, then clamp high + fp8 cast.
                    t1 = pool.tile([P, cols], F32, tag=f"qt{cols}", name=f"qt_{nm}")
                    nc.scalar.activation(
                        t1, src, ACTF.Copy, bias=MAGIC, scale=rdenb[m][:, 0:1]
                    )
                    nc.vector.tensor_scalar(t1, t1, MAGIC, -1.0, ALU.subtract, ALU.max)
                    nc.vector.tensor_scalar(dst, t1, 1.0, None, ALU.min)

                with tc.tile_pool(name="quant", bufs=2) as qpool:
                    def phase_a(m, w, rows, cols, tag, engines):
                        # partial |w| sums in [P, CD] chunks on DVE and/or ACT
                        ci = 0
                        for r in range(rows):
                            for c0 in range(0, cols, CD):
                                cw = min(CD, cols - c0)
                                st = qpool.tile(
                                    [P, CD], F16, tag=tag, bufs=3,
                                    name=f"a{m}_{r}_{c0}",
                                )
                                nc.sync.dma_start(
                                    st[:, :cw], w[r * P : (r + 1) * P, c0 : c0 + cw]
                                )
                                part = qpool.tile(
                                    [P, 1], F32, tag="sp", bufs=4,
                                    name=f"sp{m}_{r}_{c0}",
                                )
                                eng = engines[ci % len(engines)]
                                ci += 1
                                if eng == "dve":
                                    nc.vector.tensor_reduce(
                                        part, st[:, :cw], axis=AX.X, op=ALU.add,
                                        apply_absolute_value=True,
                                    )
                                else:
                                    aout = qpool.tile(
                                        [P, CD], F16, tag="ao", bufs=2,
                                        name=f"ao{m}_{r}_{c0}",
                                    )
                                    nc.scalar.activation(
                                        aout[:, :cw], st[:, :cw], ACTF.Abs,
                                        accum_out=part,
                                    )
                                nc.vector.tensor_tensor(
                                    acc[:, m : m + 1], acc[:, m : m + 1], part,
                                    op=ALU.add,
                                )
                        # global scale via tiny AllReduce
                        allb = qpool.tile([P, 1], F32, tag="allb", bufs=2, name=f"al{m}")
                        nc.gpsimd.partition_all_reduce(
                            allb, acc[:, m : m + 1], P, bass_isa.ReduceOp.add
                        )
                        nc.vector.tensor_copy(sums[0:1, m : m + 1], allb[0:1, 0:1])
                        nc.sync.dma_start(cc_in[m][:], sums[0:1, m : m + 1])
                        nc.gpsimd.collective_compute(
                            "AllReduce", ALU.add, ins=[cc_in[m][:]],
                            outs=[cc_out[m][:]], replica_groups=rg,
                        )
                        nc.sync.dma_start(gsum[m][:], cc_out[m][:])
                        nc.vector.tensor_scalar(
                            den[m], gsum[m], rn, EPS, ALU.mult, ALU.add
                        )
                        nc.vector.reciprocal(rden[m], den[m])
                        nc.vector.tensor_scalar(s3[m], gsum[m], rn, None, ALU.mult)
                        nc.gpsimd.partition_broadcast(rdenb[m], rden[m])

                    # ---- gate: quantize into the SBUF-resident tile ----
                    phase_a(0, wgT, HT, i_s, "ag", ("dve", "act"))
                    load_xb(0)
                    for h in range(HT):
                        st = qpool.tile([P, i_s], F16, tag="qs", bufs=3, name=f"qsg{h}")
                        nc.sync.dma_start(st[:], wgT[h * P : (h + 1) * P, :])
                        qround(qg_res[:, h, :], st, 0, qpool, i_s, f"g{h}")
                    load_xb(1)

                    # ---- up/down scales after B(wg) so their chunk reads don't
                    # steal DMA bandwidth from the first-matmul critical path;
                    # the ACT-side reduces still overlap B(wg)'s DVE work ----
                    phase_a(1, wuT, IT, HT * P, "ad", ("act",))
                    phase_a(2, wdT, IT, H, "ad", ("act",))

                    # ---- up: quantize to DRAM fp8. wuT arrives host-prearranged
                    # in the i-major lhsT layout, so reads AND writes are fully
                    # contiguous (the old h-major layout forced 128B-run strided
                    # writes that delayed qu_d by ~250us) ----
                    for it in range(IT):
                        for c0 in range(0, HT * P, CD):
                            st = qpool.tile(
                                [P, CD], F16, tag="ad", bufs=3, name=f"qsu{it}_{c0}"
                            )
                            nc.sync.dma_start(
                                st[:], wuT[it * P : (it + 1) * P, c0 : c0 + CD]
                            )
                            qb = qpool.tile(
                                [P, CD], FP8, tag="qbd", bufs=2, name=f"qbu{it}_{c0}"
                            )
                            qround(qb, st, 1, qpool, CD, f"u{it}_{c0}")
                            nc.sync.dma_start(qu_d[it, :, c0 : c0 + CD], qb[:])

                    # ---- down: quantize to DRAM fp8 in column chunks ----
                    for it in range(IT):
                        for c0 in range(0, H, CD):
                            st = qpool.tile(
                                [P, CD], F16, tag="ad", bufs=3, name=f"qsd{it}_{c0}"
                            )
                            nc.sync.dma_start(
                                st[:], wdT[it * P : (it + 1) * P, c0 : c0 + CD]
                            )
                            qb = qpool.tile(
                                [P, CD], FP8, tag="qbd", bufs=2, name=f"qbd{it}_{c0}"
                            )
                            qround(qb, st, 2, qpool, CD, f"d{it}_{c0}")
                            nc.sync.dma_start(qd_d[it, :, c0 : c0 + CD], qb[:])

                    nc.vector.tensor_tensor(cp01, s3[0], s3[1], op=ALU.mult)
                    nc.vector.tensor_tensor(cprod, cp01, s3[2], op=ALU.mult)
                    nc.gpsimd.partition_broadcast(cb, cprod)

                # ---------- Phase C: software-pipelined block loop ----------
                def gate_phase(b):
                    xb = xb_res[b % 2]
                    for i in range(IT):
                        pg = pspool.tile([P, TB], F32, tag="ps", name=f"pg{b}_{i}")
                        for h in range(HT):
                            nc.tensor.matmul(
                                pg,
                                lhsT=qg_res[:, h, i * P : (i + 1) * P],
                                rhs=xb[:, h, :],
                                start=(h == 0),
                                stop=(h == HT - 1),
                            )
                        nc.scalar.activation(gT[b % 2][:, i, :], pg, ACTF.Copy)

                with tc.tile_pool(name="main", bufs=2) as mpool:
                    gate_phase(0)
                    for b in range(NB):
                        # issue the first up-weight stream DMAs (one per buffer)
                        # ahead of the gate matmul burst so they beat the
                        # per-queue backlog without head-of-line blocking
                        qucs = {}
                        for i in range(3):
                            qucs[i] = rpool.tile(
                                [P, HT * P], FP8, tag="quc", bufs=3, name=f"quc{b}_{i}"
                            )
                            nc.sync.dma_start(qucs[i][:], qu_d[i])
                        if b + 1 < NB:
                            gate_phase(b + 1)
                        xb = xb_res[b % 2]
                        interT = mpool.tile(
                            [P, IT, TB], BF16, tag="inter", bufs=1, name=f"int{b}"
                        )
                        for i in range(IT):
                            if i in qucs:
                                quc = qucs[i]
                            else:
                                quc = rpool.tile(
                                    [P, HT * P], FP8, tag="quc", bufs=3,
                                    name=f"quc{b}_{i}",
                                )
                                nc.sync.dma_start(quc[:], qu_d[i])
                            pu = pspool.tile([P, TB], F32, tag="ps", name=f"pu{b}_{i}")
                            for h in range(HT):
                                nc.tensor.matmul(
                                    pu,
                                    lhsT=quc[:, h * P : (h + 1) * P],
                                    rhs=xb[:, h, :],
                                    start=(h == 0),
                                    stop=(h == HT - 1),
                                )
                            nc.vector.tensor_tensor(
                                interT[:, i, :], pu, gT[b % 2][:, i, :], op=ALU.mult
                            )
                        if b + 2 < NB:
                            load_xb(b + 2)
                        for hb in range(HB):
                            qdc = rpool.tile(
                                [P, IT, TB], FP8, tag="qdc", bufs=3, name=f"qdc{b}_{hb}"
                            )
                            nc.sync.dma_start(
                                qdc[:],
                                qd_d[:, :, hb * TB : (hb + 1) * TB].rearrange(
                                    "i p f -> p i f"
                                ),
                            )
                            pos = [
                                pspool.tile(
                                    [P, TB], F32, tag="ps", name=f"po{b}_{hb}_{t}"
                                )
                                for t in range(TS)
                            ]
                            for i in range(IT):
                                for ts in range(TS):
                                    nc.tensor.matmul(
                                        pos[ts],
                                        lhsT=interT[:, i, ts * P : (ts + 1) * P],
                                        rhs=qdc[:, i, :],
                                        start=(i == 0),
                                        stop=(i == IT - 1),
                                    )
                            ob = mpool.tile(
                                [P, TS, TB], BF16, tag="ob", bufs=2, name=f"ob{b}_{hb}"
                            )
                            for ts in range(TS):
                                nc.vector.tensor_scalar(
                                    ob[:, ts, :], pos[ts], cb[:, 0:1], None, ALU.mult
                                )
                            if b == NB - 1:
                                hpb = HB // 2  # hb blocks per half
                                nc.sync.dma_start(
                                    outbL[hb // hpb][
                                        :, (hb % hpb) * TB : (hb % hpb + 1) * TB
                                    ].rearrange("(g p) f -> p g f", p=P),
                                    ob[:],
                                )
                                # fire each half's RS as soon as it is complete
                                if hb % hpb == hpb - 1:
                                    k = hb // hpb
                                    h2 = H // 2
                                    nc.gpsimd.collective_compute(
                                        "ReduceScatter",
                                        ALU.add,
                                        ins=[outbL[k][:]],
                                        outs=[rsbL[k][:]],
                                        replica_groups=rg,
                                    )
                                    nc.sync.dma_start(
                                        y[b, :, k * h2 : (k + 1) * h2], rsbL[k][:]
                                    )
                            else:
                                nc.sync.dma_start(
                                    outb[b][:, hb * TB : (hb + 1) * TB].rearrange(
                                        "(g p) f -> p g f", p=P
                                    ),
                                    ob[:],
                                )
                        if b != NB - 1:
                            nc.gpsimd.collective_compute(
                                "ReduceScatter",
                                ALU.add,
                                ins=[outb[b][:]],
                                outs=[rsb[b][:]],
                                replica_groups=rg,
                            )
                            nc.sync.dma_start(y[b], rsb[b][:])
    nc.compile()
    return nc


_NC_CACHE = {}


def _get_nc(T, H, I_real, n_cores):
    key = (T, H, I_real, n_cores)
    if key not in _NC_CACHE:
        _NC_CACHE[key] = build_bass(T, H, I_real, n_cores)
    return _NC_CACHE[key]


def shard_inputs(hidden_states, w_gate, w_up, w_down, n_cores=N_CORES):
    """Host prep: flatten/transpose/zero-pad/slice; activations cast to bf16,
    weights to f16 (scale + ternarization still computed on device)."""
    B, S, H = hidden_states.shape
    T = B * S
    I_real = w_gate.shape[0]
    i_s, _ = shard_sizes(I_real, n_cores)
    Ip = i_s * n_cores
    bf16 = mybir.dt.np(BF16)

    xTb = np.ascontiguousarray(
        hidden_states.reshape(T, H).T.astype(np.float32, copy=False)
    ).astype(bf16)
    wgT = np.zeros((H, Ip), np.float16)
    wgT[:, :I_real] = w_gate.T
    wuT = np.zeros((H, Ip), np.float16)
    wuT[:, :I_real] = w_up.T
    wdT = np.zeros((Ip, H), np.float16)
    wdT[:I_real, :] = w_down.T

    HT, IT = H // P, i_s // P
    in_maps = []
    for c in range(n_cores):
        wu_c = wuT[:, c * i_s : (c + 1) * i_s]  # [H, i_s]
        # i-major up-lhsT layout: [i, p, h, f] <- [h*P+p, i*P+f]
        wu_dev = np.ascontiguousarray(
            wu_c.reshape(HT, P, IT, P).transpose(2, 1, 0, 3).reshape(i_s, H)
        )
        in_maps.append(
            {
                "xTb": xTb,
                "wgT": np.ascontiguousarray(wgT[:, c * i_s : (c + 1) * i_s]),
                "wuT": wu_dev,
                "wdT": np.ascontiguousarray(wdT[c * i_s : (c + 1) * i_s, :]),
            }
        )
    return in_maps, (B, S, H, T)


def kernel(hidden_states, w_gate, w_up, w_down, _trace=False):
    global LAST_RESULTS
    n_cores = N_CORES
    in_maps, (B, S, H, T) = shard_inputs(hidden_states, w_gate, w_up, w_down, n_cores)
    I_real = w_gate.shape[0]
    nc = _get_nc(T, H, I_real, n_cores)
    res = run_bass_kernel_spmd(
        nc, in_maps, core_ids=list(range(n_cores)), trace=_trace
    )
    LAST_RESULTS = res

    NB = T // TB
    rq = TB // n_cores
    out = np.empty((T, H), np.float32)
    for c in range(n_cores):
        yc = res.results[c]["y"]  # [NB, rq, H] bf16
        for b in range(NB):
            out[b * TB + c * rq : b * TB + (c + 1) * rq] = yc[b].astype(np.float32)
    return out.reshape(B, S, H)
